# revision 14
# baseline (speedup 1.0000x reference)
"""DeepCrossing (embedding lookup + 3 residual MLP blocks + sigmoid) on 8 TRN2 NeuronCores.

Data-parallel: batch 16384 is split into 8 shards of 2048 rows; the embedding
table (stored bf16, flattened to [26*100001, 16]) and MLP weights are
replicated to every core.  Per core:

  - the identity tile is built first so gpsimd descriptor generation for the
    gathers does not block it,
  - one up-front DMA brings all gather indices into SBUF (scalar-engine HWDGE
    ring, separate from the weight blob's ring), then 4 batched indirect DMAs
    (512 samples each) gather 26 embedding rows per sample into sample-major
    [128, 4*416] bf16 tiles,
  - all MLP weights ship as a single host-packed [128, COLS] bf16 blob so the
    weight DMA uses >=4KB descriptors (line rate),
  - PE transposes convert gathered tiles to feature-major K-tiles [*, 2048]
    (full batch width per core) so all matmuls run with features on
    partitions; 4 transposes share one PSUM tile so each psum->sbuf copy
    moves 512 columns,
  - 3 residual blocks: h = x@w1+b1 ; x = relu(h@w2+b2+x).  Matmul inputs are
    bf16 with fp32 PSUM accumulation; the loop order is weights-outer /
    chunk-inner so each LDWEIGHTS is reused for 4 matmuls; pointwise ops run
    full-width ([*, 2048]) to amortize per-instruction overhead: residual add
    on DVE, bias+relu / bias-copy / sigmoid on the scalar engine,
  - all PSUM tiles share one pool tag (2 slots x 4 banks = all of PSUM).
"""

import sys

import numpy as np
import ml_dtypes

for _p in ("/opt/trn_rl_repo",):
    if _p not in sys.path:
        sys.path.insert(0, _p)

import concourse.bass as bass
import concourse.bacc as bacc
import concourse.mybir as mybir
import concourse.tile as tile
from concourse.bass_utils import run_bass_kernel_spmd
from concourse.masks import make_identity

# Problem constants (hardcoded per the task contract).
B = 16384
N_CORES = 8
BC = B // N_CORES          # 2048 rows per core
N_SPARSE = 26
N_DENSE = 13
EMB_DIM = 16
VOCAB1 = 100001            # VOCAB + 1 rows per table
HIDDEN = 256
N_BLOCKS = 3
STACK = N_SPARSE * EMB_DIM + N_DENSE   # 429
FD = N_SPARSE * EMB_DIM                # 416

CHUNK = 512                # columns per matmul (PSUM bank limit)
N_CHUNKS = BC // CHUNK     # 4
P = 128
N_STILES = BC // P         # 16 sample tiles of 128

# K-tiles over the STACK dimension: 128+128+128+45.
KT = [(0, 128), (128, 128), (256, 128), (384, STACK - 384)]
# Embedding features inside each K-tile (tile 3 has 32 emb + 13 dense rows).
KT_EMB = [128, 128, 128, FD - 384]

F32 = mybir.dt.float32
BF16 = mybir.dt.bfloat16
I32 = mybir.dt.int32

# ── weight blob column layout (bf16, [128, WCOLS]) ──────────────────────
_off = 0
W1_OFF = []          # [block][kt] -> col
for _i in range(N_BLOCKS):
    W1_OFF.append([])
    for _kt in range(len(KT)):
        W1_OFF[_i].append(_off)
        _off += HIDDEN
W2_OFF = []          # [block][kh] -> col
for _i in range(N_BLOCKS):
    W2_OFF.append([])
    for _kh in range(2):
        W2_OFF[_i].append(_off)
        _off += STACK
WL_OFF = []          # [kt] -> col
for _kt in range(len(KT)):
    WL_OFF.append(_off)
    _off += 1
WCOLS = _off

# bias blob (f32, [128, BCOLS])
B1_COL = lambda i, mt: i * 2 + mt
B2_COL = lambda i, kt: 6 + i * 4 + kt
BL_COL = 18
BCOLS = 19

_compiled = {}
DEBUG_X = False


def _build_bass():
    nc = bacc.Bacc()

    tbl = nc.declare_dram_parameter("tbl", [N_SPARSE * VOCAB1, EMB_DIM], BF16, False)
    idx = nc.declare_dram_parameter("idx", [P, N_STILES * N_SPARSE], I32, False)
    dnt = nc.declare_dram_parameter("dnt", [N_DENSE, BC], BF16, False)
    wb = nc.declare_dram_parameter("wb", [P, WCOLS], BF16, False)
    bb = nc.declare_dram_parameter("bb", [P, BCOLS], F32, False)
    out = nc.declare_dram_parameter("out", [1, BC], F32, True)
    dbg = nc.declare_dram_parameter("dbg", [len(KT), P, BC], BF16, True) \
        if DEBUG_X else None
    dbgg = nc.declare_dram_parameter("dbgg", [N_CHUNKS, P, 4 * FD], BF16, True) \
        if DEBUG_X else None

    with tile.TileContext(nc) as tc:
        with (
            tc.tile_pool(name="const", bufs=1) as cp,
            tc.tile_pool(name="wp", bufs=1) as wp,
            tc.tile_pool(name="gat", bufs=N_CHUNKS) as gp,
            tc.tile_pool(name="xp", bufs=2) as xp,
            tc.tile_pool(name="hp", bufs=2) as hp,
            tc.tile_pool(name="outp", bufs=1) as op_,
            tc.tile_pool(name="ps", bufs=2, space="PSUM") as ps,
        ):
            # identity first: it is gpsimd work and everything PE does waits
            # on it; the gather descriptor generation queues behind it.
            ident = cp.tile([P, P], BF16)
            make_identity(nc, ident[:])

            # all gather indices in one early DMA on the ACT HWDGE ring
            it = cp.tile([P, N_STILES * N_SPARSE], I32)
            nc.scalar.dma_start(out=it[:], in_=idx[:])

            # Per-field gathers: the HW indirect DMA consumes exactly ONE
            # index per partition-row descriptor (multi-index offset APs pair
            # descriptors with indices non-deterministically), so each DMA
            # gathers one field for one 128-sample tile: idx [128, 1] ->
            # out [128, 16].
            xg = []
            for c in range(N_CHUNKS):
                g = gp.tile([P, 4 * FD], BF16, tag="xg", name=f"xg_{c}")
                for j in range(4):
                    st = c * 4 + j
                    for f in range(N_SPARSE):
                        nc.gpsimd.indirect_dma_start(
                            out=g[:, j * FD + f * EMB_DIM:
                                  j * FD + (f + 1) * EMB_DIM],
                            out_offset=None,
                            in_=tbl[:],
                            in_offset=bass.IndirectOffsetOnAxis(
                                ap=it[:, st * N_SPARSE + f:
                                      st * N_SPARSE + f + 1], axis=0),
                        )
                xg.append(g)

            # weights: one big line-rate DMA each
            wsb = wp.tile([P, WCOLS], BF16)
            nc.sync.dma_start(out=wsb[:], in_=wb[:])
            bsb = wp.tile([P, BCOLS], F32)
            nc.sync.dma_start(out=bsb[:], in_=bb[:])

            out_sb = op_.tile([1, BC], F32)

            # --- gather -> feature-major K-tiles [kn, 2048] ---
            xcur = []
            for kt, (k0, kn) in enumerate(KT):
                xcur.append(xp.tile([kn, BC], BF16, tag=f"x_{kt}",
                                    name=f"x_{kt}"))
            # dense rows ride in the tail of K-tile 3 (host pre-transposed)
            nc.scalar.dma_start(out=xcur[3][KT_EMB[3]:, :], in_=dnt[:])

            for kt, (k0, kn) in enumerate(KT):
                ke = KT_EMB[kt]
                for c in range(N_CHUNKS):
                    pt = ps.tile([P, CHUNK], BF16, tag="ps", name="pt")
                    for j in range(4):
                        nc.tensor.transpose(
                            out=pt[:ke, j * P:(j + 1) * P],
                            in_=xg[c][:, j * FD + k0:j * FD + k0 + ke],
                            identity=ident[:])
                    nc.vector.tensor_copy(
                        out=xcur[kt][:ke, c * CHUNK:(c + 1) * CHUNK],
                        in_=pt[:ke, :])

            if DEBUG_X:
                for kt, (k0, kn) in enumerate(KT):
                    nc.sync.dma_start(out=dbg[kt, :kn, :], in_=xcur[kt][:])
                for c in range(N_CHUNKS):
                    nc.sync.dma_start(out=dbgg[c], in_=xg[c][:])

            # --- residual MLP blocks (weights-outer, chunk-inner) ---
            for i in range(N_BLOCKS):
                hbf = []
                for mt in range(2):
                    ph = ps.tile([P, BC], F32, tag="ps", name="ph")
                    for kt, (k0, kn) in enumerate(KT):
                        for c in range(N_CHUNKS):
                            nc.tensor.matmul(
                                ph[:, c * CHUNK:(c + 1) * CHUNK],
                                lhsT=wsb[:kn, W1_OFF[i][kt] + mt * P:
                                         W1_OFF[i][kt] + mt * P + P],
                                rhs=xcur[kt][:, c * CHUNK:(c + 1) * CHUNK],
                                start=(kt == 0), stop=(kt == len(KT) - 1),
                            )
                    ht = hp.tile([P, BC], BF16, tag=f"h_{mt}")
                    nc.scalar.activation(
                        out=ht[:], in_=ph[:],
                        func=mybir.ActivationFunctionType.Identity,
                        bias=bsb[:, B1_COL(i, mt):B1_COL(i, mt) + 1],
                    )
                    hbf.append(ht)

                xnew = []
                for kt, (k0, kn) in enumerate(KT):
                    px = ps.tile([P, BC], F32, tag="ps", name="px")
                    for kh in range(2):
                        for c in range(N_CHUNKS):
                            nc.tensor.matmul(
                                px[:kn, c * CHUNK:(c + 1) * CHUNK],
                                lhsT=wsb[:, W2_OFF[i][kh] + k0:
                                         W2_OFF[i][kh] + k0 + kn],
                                rhs=hbf[kh][:, c * CHUNK:(c + 1) * CHUNK],
                                start=(kh == 0), stop=(kh == 1),
                            )
                    # z = h@w2 + x  (residual add on DVE), then
                    # x' = relu(z + b2) on the scalar engine
                    zt = hp.tile([kn, BC], F32, tag="z")
                    nc.vector.tensor_add(zt[:], px[:kn, :], xcur[kt][:])
                    xt = xp.tile([kn, BC], BF16, tag=f"x_{kt}")
                    nc.scalar.activation(
                        out=xt[:], in_=zt[:],
                        func=mybir.ActivationFunctionType.Relu,
                        bias=bsb[:kn, B2_COL(i, kt):B2_COL(i, kt) + 1],
                    )
                    xnew.append(xt)
                xcur = xnew

            # --- head: w_last + sigmoid ---
            po = ps.tile([1, BC], F32, tag="ps", name="po")
            for c in range(N_CHUNKS):
                for kt, (k0, kn) in enumerate(KT):
                    nc.tensor.matmul(
                        po[:, c * CHUNK:(c + 1) * CHUNK],
                        lhsT=wsb[:kn, WL_OFF[kt]:WL_OFF[kt] + 1],
                        rhs=xcur[kt][:, c * CHUNK:(c + 1) * CHUNK],
                        start=(kt == 0), stop=(kt == len(KT) - 1))
            nc.scalar.activation(
                out=out_sb[:], in_=po[:],
                func=mybir.ActivationFunctionType.Sigmoid,
                bias=bsb[0:1, BL_COL:BL_COL + 1],
            )

            nc.sync.dma_start(out=out[:], in_=out_sb[:])

    nc.finalize()
    return nc


def _prep_shared(emb_tables, w1s, b1s, w2s, b2s, w_last, b_last):
    bf = ml_dtypes.bfloat16
    tbl = np.ascontiguousarray(
        np.asarray(emb_tables, dtype=np.float32).reshape(N_SPARSE * VOCAB1, EMB_DIM)
    ).astype(bf)

    w1s = np.asarray(w1s, dtype=np.float32)
    w2s = np.asarray(w2s, dtype=np.float32)
    w_last = np.asarray(w_last, dtype=np.float32)

    wb = np.zeros((P, WCOLS), dtype=np.float32)
    for i in range(N_BLOCKS):
        for kt, (k0, kn) in enumerate(KT):
            wb[:kn, W1_OFF[i][kt]:W1_OFF[i][kt] + HIDDEN] = w1s[i, k0:k0 + kn, :]
        for kh in range(2):
            wb[:, W2_OFF[i][kh]:W2_OFF[i][kh] + STACK] = \
                w2s[i, kh * P:(kh + 1) * P, :]
    for kt, (k0, kn) in enumerate(KT):
        wb[:kn, WL_OFF[kt]:WL_OFF[kt] + 1] = w_last[k0:k0 + kn, :]

    bbl = np.zeros((P, BCOLS), dtype=np.float32)
    b1s = np.asarray(b1s, dtype=np.float32)
    b2s = np.asarray(b2s, dtype=np.float32)
    for i in range(N_BLOCKS):
        for mt in range(2):
            bbl[:, B1_COL(i, mt)] = b1s[i, mt * P:(mt + 1) * P]
        for kt, (k0, kn) in enumerate(KT):
            bbl[:kn, B2_COL(i, kt)] = b2s[i, k0:k0 + kn]
    bbl[0, BL_COL] = np.asarray(b_last, dtype=np.float32).reshape(-1)[0]

    return {"tbl": tbl, "wb": wb.astype(bf), "bb": bbl}


def kernel(dense, sparse, label, emb_tables, w1s, b1s, w2s, b2s, w_last, b_last,
           **_unused):
    if "nc" not in _compiled:
        _compiled["nc"] = _build_bass()
    nc = _compiled["nc"]

    shared = _prep_shared(emb_tables, w1s, b1s, w2s, b2s, w_last, b_last)

    dense = np.asarray(dense, dtype=np.float32)
    sparse = np.asarray(sparse)
    flat_idx = (sparse.astype(np.int64)
                + (np.arange(N_SPARSE, dtype=np.int64) * VOCAB1)[None, :]
                ).astype(np.int32)
    dnt = np.asarray(dense.T, dtype=np.float32).astype(ml_dtypes.bfloat16)  # [13, B]

    in_maps = []
    for i in range(N_CORES):
        s = slice(i * BC, (i + 1) * BC)
        m = dict(shared)
        # [BC, 26] -> [16, 128, 26] -> [128, 16*26]  (idx[p, j*26+f])
        m["idx"] = np.ascontiguousarray(
            flat_idx[s].reshape(N_STILES, P, N_SPARSE)
            .transpose(1, 0, 2).reshape(P, N_STILES * N_SPARSE))
        m["dnt"] = np.ascontiguousarray(dnt[:, s])
        in_maps.append(m)

    res = run_bass_kernel_spmd(nc, in_maps, core_ids=list(range(N_CORES)))
    _compiled["last_results"] = res
    preds = np.concatenate(
        [np.asarray(r["out"], dtype=np.float32).reshape(BC, 1) for r in res.results],
        axis=0,
    )
    return preds, np.asarray(label)
